# revision 7
# baseline (speedup 1.0000x reference)
"""Trainium2 Bass kernel for the MetaNeuralCV model (dense_mlp).

Math (per sample x, score s; MLP 8 -> 256 -> 256 -> 1 -> 8):
    z0 = W0 x + b0;  y0 = tanh(z0)
    z1 = W1 y0 + b1; y1 = tanh(z1)
    z2 = w2.y1 + b2; y2 = tanh(z2)        (w2 = W2[0])
    u  = y2 * w3 + b3                      (w3 = W3[:,0])
    out = c + trace(J) + u.s
The last two layers pass through scalar y2, so the Jacobian is rank-1:
    trace(J) = (1 - y2^2) * q,
    q = w2^T D1 W1 D0 (W0 w3) = sum_i d1_i * stilde_i
with D# = diag(1 - y#^2),  a = W0 w3,  Wu = diag(w2) W1 diag(a),
    stilde = Wu (1 - y0^2) = r1 - Wu y0^2   (r1 = Wu @ ones)
    q = R1 - sum_i y1_i^2 r1_i + sum_i (y1_i^2 - 1) * (Wu y0^2)_i
and u.s = y2 * (w3.s) + (b3.s).

Sharding: pure data parallel over batch across 8 cores; weights replicated.
Layout on device: features on partitions, batch on the free dim.  Inputs are
pre-transposed on the host to [8, B/8] so DMA loads are contiguous.
"""

import numpy as np
import ml_dtypes

import concourse.bass as bass
import concourse.mybir as mybir
import concourse.tile as tile
from concourse import bacc
from concourse.bass_utils import run_bass_kernel_spmd

B_TOTAL = 65536
D_IN = 8
H = 256
N_CORES = 8
BC = B_TOTAL // N_CORES        # 8192 samples per core
NT = 512                       # batch tile (one fp32 PSUM bank)
NTILES = BC // NT              # 16
FB = BC // 128                 # 64: free dim of [128, FB] staging layout

F32 = mybir.dt.float32
F32R = mybir.dt.float32r
BF16 = mybir.dt.bfloat16

# Filled by kernel() so test.py can report HW exec time.
LAST_RESULT = None


def _build(b2f: float, cf: float, R1f: float):
    nc = bacc.Bacc("TRN2", target_bir_lowering=False, debug=False)
    Tanh = mybir.ActivationFunctionType.Tanh
    Alu = mybir.AluOpType

    xT = nc.dram_tensor("xT", [D_IN, BC], F32R, kind="ExternalInput")
    sT = nc.dram_tensor("sT", [D_IN, BC], F32R, kind="ExternalInput")
    w0t_d = nc.dram_tensor("w0t", [D_IN, H], F32R, kind="ExternalInput")
    w1tA_d = nc.dram_tensor("w1tA", [128, H], BF16, kind="ExternalInput")
    w1tB_d = nc.dram_tensor("w1tB", [128, H], BF16, kind="ExternalInput")
    wutA_d = nc.dram_tensor("wutA", [128, H], BF16, kind="ExternalInput")
    wutB_d = nc.dram_tensor("wutB", [128, H], BF16, kind="ExternalInput")
    w2c_d = nc.dram_tensor("w2c", [128, 2], BF16, kind="ExternalInput")
    r1c_d = nc.dram_tensor("r1c", [128, 2], BF16, kind="ExternalInput")
    onec_d = nc.dram_tensor("onec", [128, 1], BF16, kind="ExternalInput")
    b0c_d = nc.dram_tensor("b0c", [128, 2], F32, kind="ExternalInput")
    b1c_d = nc.dram_tensor("b1c", [128, 2], F32, kind="ExternalInput")
    swc_d = nc.dram_tensor("swc", [D_IN, 2], F32R, kind="ExternalInput")
    b2c_d = nc.dram_tensor("b2c", [128, 1], F32, kind="ExternalInput")
    out_d = nc.dram_tensor("out", [BC], F32, kind="ExternalOutput")

    with tile.TileContext(nc) as tc:
        with (
            tc.tile_pool(name="const", bufs=1) as cp,
            tc.tile_pool(name="work", bufs=2) as wp,
            tc.tile_pool(name="stage", bufs=1) as stp,
            tc.tile_pool(name="ps_fw", bufs=1, space="PSUM") as pfw,
            tc.tile_pool(name="ps_u", bufs=1, space="PSUM") as pu,
            tc.tile_pool(name="ps_r", bufs=1, space="PSUM") as pr,
            tc.tile_pool(name="ps_p", bufs=1, space="PSUM") as pp,
        ):
            xts = cp.tile([D_IN, BC], F32R)
            nc.sync.dma_start(out=xts[:], in_=xT[:])
            sts = cp.tile([D_IN, BC], F32R)
            nc.sync.dma_start(out=sts[:], in_=sT[:])
            w0s = cp.tile([D_IN, H], F32R)
            nc.sync.dma_start(out=w0s[:], in_=w0t_d[:])
            w1sA = cp.tile([128, H], BF16)
            nc.sync.dma_start(out=w1sA[:], in_=w1tA_d[:])
            w1sB = cp.tile([128, H], BF16)
            nc.sync.dma_start(out=w1sB[:], in_=w1tB_d[:])
            wusA = cp.tile([128, H], BF16)
            nc.sync.dma_start(out=wusA[:], in_=wutA_d[:])
            wusB = cp.tile([128, H], BF16)
            nc.sync.dma_start(out=wusB[:], in_=wutB_d[:])
            w2s = cp.tile([128, 2], BF16)
            nc.sync.dma_start(out=w2s[:], in_=w2c_d[:])
            r1s = cp.tile([128, 2], BF16)
            nc.sync.dma_start(out=r1s[:], in_=r1c_d[:])
            ones1 = cp.tile([128, 1], BF16)
            nc.sync.dma_start(out=ones1[:], in_=onec_d[:])
            b0s = cp.tile([128, 2], F32)
            nc.sync.dma_start(out=b0s[:], in_=b0c_d[:])
            b1s = cp.tile([128, 2], F32)
            nc.sync.dma_start(out=b1s[:], in_=b1c_d[:])
            sws = cp.tile([D_IN, 2], F32R)
            nc.sync.dma_start(out=sws[:], in_=swc_d[:])
            b2s = cp.tile([128, 1], F32)
            nc.sync.dma_start(out=b2s[:], in_=b2c_d[:])

            # Staging for the per-sample scalars, laid out [128, FB] so the
            # tail runs wide across partitions (batch index = p*FB + f).
            z2s = stp.tile([128, FB], F32)
            s1s = stp.tile([128, FB], F32)
            s0s = stp.tile([128, FB], F32)
            p0s = stp.tile([128, FB], F32)
            p1s = stp.tile([128, FB], F32)

            for t in range(NTILES):
                ns = bass.ts(t, NT)

                # ---- layer 0: z0 = W0 x + b0 (K=8, f32r) ----
                z0 = pfw.tile([128, 2, NT], F32, tag="z0")
                for h in (0, 1):
                    nc.tensor.matmul(
                        z0[:, h, :], w0s[:, bass.ts(h, 128)], xts[:, ns],
                        start=True, stop=True,
                    )
                y0 = wp.tile([128, 2, NT], BF16, tag="y0")
                for h in (0, 1):
                    nc.scalar.activation(
                        y0[:, h, :], z0[:, h, :], Tanh, bias=b0s[:, h:h + 1]
                    )

                # ---- layer 1: z1 = W1 y0 + b1 (K=256, bf16) ----
                z1 = pfw.tile([128, 2, NT], F32, tag="z1")
                for m in (0, 1):
                    nc.tensor.matmul(
                        z1[:, m, :], w1sA[:, bass.ts(m, 128)], y0[:, 0, :],
                        start=True, stop=False,
                    )
                    nc.tensor.matmul(
                        z1[:, m, :], w1sB[:, bass.ts(m, 128)], y0[:, 1, :],
                        start=False, stop=True,
                    )
                y1 = wp.tile([128, 2, NT], BF16, tag="y1")
                for h in (0, 1):
                    nc.scalar.activation(
                        y1[:, h, :], z1[:, h, :], Tanh, bias=b1s[:, h:h + 1]
                    )

                # ---- squares (DVE, bf16 2x mode) ----
                sq0 = wp.tile([128, 2, NT], BF16, tag="sq0")
                for h in (0, 1):
                    nc.vector.tensor_mul(sq0[:, h, :], y0[:, h, :], y0[:, h, :])
                sq1 = wp.tile([128, 2, NT], BF16, tag="sq1")
                for h in (0, 1):
                    nc.vector.tensor_mul(sq1[:, h, :], y1[:, h, :], y1[:, h, :])

                # ---- u = Wu y0^2 (K=256, bf16) ----
                u = pu.tile([128, 2, NT], F32, tag="u")
                for m in (0, 1):
                    nc.tensor.matmul(
                        u[:, m, :], wusA[:, bass.ts(m, 128)], sq0[:, 0, :],
                        start=True, stop=False,
                    )
                    nc.tensor.matmul(
                        u[:, m, :], wusB[:, bass.ts(m, 128)], sq0[:, 1, :],
                        start=False, stop=True,
                    )

                # ---- wpp = (y1^2 - 1) * u ----
                wpp = wp.tile([128, 2, NT], BF16, tag="wpp")
                for h in (0, 1):
                    nc.vector.scalar_tensor_tensor(
                        wpp[:, h, :], sq1[:, h, :], 1.0, u[:, h, :],
                        op0=Alu.subtract, op1=Alu.mult,
                    )

                # ---- partition reductions on PE (one PSUM bank, 4 col groups)
                red = pr.tile([128, NT], F32, tag="red")
                nc.tensor.matmul(red[0:1, :], w2s[:, 0:1], y1[:, 0, :],
                                 start=True, stop=False, tile_position=(0, 0))
                nc.tensor.matmul(red[0:1, :], w2s[:, 1:2], y1[:, 1, :],
                                 start=False, stop=True, tile_position=(0, 0))
                nc.tensor.matmul(red[32:33, :], r1s[:, 0:1], sq1[:, 0, :],
                                 start=True, stop=False, tile_position=(0, 32))
                nc.tensor.matmul(red[32:33, :], r1s[:, 1:2], sq1[:, 1, :],
                                 start=False, stop=True, tile_position=(0, 32))
                nc.tensor.matmul(red[64:65, :], ones1[:, 0:1], wpp[:, 0, :],
                                 start=True, stop=False, tile_position=(0, 64))
                nc.tensor.matmul(red[64:65, :], ones1[:, 0:1], wpp[:, 1, :],
                                 start=False, stop=True, tile_position=(0, 64))
                # f32r forbids non-zero tile_position -> own PSUM bank
                pps = pp.tile([2, NT], F32, tag="pps")
                nc.tensor.matmul(pps[0:2, :], sws[:, :], sts[:, ns],
                                 start=True, stop=True)

                # one wide PSUM->SBUF copy (cost = free dim, lanes parallel),
                # alternating engines to balance ACT/DVE load
                redsb = wp.tile([128, NT], F32, tag="redsb")
                if t % 2 == 0:
                    nc.scalar.activation(
                        redsb[0:66, :], red[0:66, :],
                        mybir.ActivationFunctionType.Copy,
                    )
                    nc.vector.tensor_copy(redsb[96:98, :], pps[0:2, :])
                else:
                    nc.vector.tensor_copy(redsb[0:66, :], red[0:66, :])
                    nc.scalar.activation(
                        redsb[96:98, :], pps[0:2, :],
                        mybir.ActivationFunctionType.Copy,
                    )

                # scatter the reduce rows into [128, FB] staging
                for row, dst in ((0, z2s), (32, s1s), (64, s0s),
                                 (96, p0s), (97, p1s)):
                    nc.sync.dma_start(
                        out=dst[t * 8:(t + 1) * 8, :], in_=redsb[row:row + 1, :]
                    )

            # ---- tail: out = c + (1-y2^2)*q + y2*P0 + P1 ----
            y2 = stp.tile([128, FB], F32)
            nc.scalar.activation(y2[:], z2s[:], Tanh, bias=b2s[:, 0:1])
            q = stp.tile([128, FB], F32)
            nc.vector.tensor_sub(q[:], s0s[:], s1s[:])
            t0 = stp.tile([128, FB], F32)
            nc.vector.tensor_mul(t0[:], y2[:], y2[:])         # y2^2
            d2 = stp.tile([128, FB], F32)
            nc.vector.tensor_scalar(d2[:], t0[:], -1.0, 1.0,
                                    op0=Alu.mult, op1=Alu.add)  # 1 - y2^2
            qq = stp.tile([128, FB], F32)
            nc.vector.tensor_scalar_add(qq[:], q[:], R1f)      # q = R1-s1+s0
            dv = stp.tile([128, FB], F32)
            nc.vector.tensor_mul(dv[:], d2[:], qq[:])          # trace(J)
            h1 = stp.tile([128, FB], F32)
            nc.vector.tensor_mul(h1[:], y2[:], p0s[:])         # y2 * (w3.s)
            o1 = stp.tile([128, FB], F32)
            nc.vector.tensor_add(o1[:], dv[:], h1[:])
            o2 = stp.tile([128, FB], F32)
            nc.vector.tensor_add(o2[:], o1[:], p1s[:])
            o3 = stp.tile([128, FB], F32)
            nc.vector.tensor_scalar_add(o3[:], o2[:], cf)
            nc.sync.dma_start(
                out=out_d.rearrange("(p f) -> p f", p=128), in_=o3[:]
            )

    nc.compile()
    return nc


def build_for_inputs(x_batch, scores_x_batch, W0, b0, W1, b1, W2, b2, W3, b3,
                     c):
    f = np.float32
    bf = ml_dtypes.bfloat16
    x = np.asarray(x_batch, f)
    s = np.asarray(scores_x_batch, f)
    W0 = np.asarray(W0, f)
    W1 = np.asarray(W1, f)
    W2 = np.asarray(W2, f)
    W3 = np.asarray(W3, f)
    b0 = np.asarray(b0, f)
    b1 = np.asarray(b1, f)
    b3 = np.asarray(b3, f)
    b2f = float(np.asarray(b2, f).reshape(-1)[0])
    cf = float(np.asarray(c, f).reshape(-1)[0])

    w2 = W2[0]
    w3 = W3[:, 0]
    a = (W0 @ w3).astype(f)
    Wu = (w2[:, None] * W1 * a[None, :]).astype(f)   # diag(w2) W1 diag(a)
    r1 = Wu.sum(axis=1).astype(f)
    R1f = float(r1.sum())

    def cols(v):
        return np.ascontiguousarray(np.stack([v[0:128], v[128:256]], axis=1))

    common = {
        "w0t": np.ascontiguousarray(W0.T),
        "w1tA": np.ascontiguousarray(W1.T[0:128]).astype(bf),
        "w1tB": np.ascontiguousarray(W1.T[128:256]).astype(bf),
        "wutA": np.ascontiguousarray(Wu.T[0:128]).astype(bf),
        "wutB": np.ascontiguousarray(Wu.T[128:256]).astype(bf),
        "w2c": cols(w2).astype(bf),
        "r1c": cols(r1).astype(bf),
        "onec": np.ones([128, 1], bf),
        "b0c": cols(b0),
        "b1c": cols(b1),
        "swc": np.ascontiguousarray(np.stack([w3, b3], axis=1)),
        "b2c": np.full([128, 1], b2f, f),
    }

    nc = _build(b2f, cf, R1f)

    in_maps = []
    for i in range(N_CORES):
        m = dict(common)
        sl = slice(i * BC, (i + 1) * BC)
        m["xT"] = np.ascontiguousarray(x[sl].T)
        m["sT"] = np.ascontiguousarray(s[sl].T)
        in_maps.append(m)

    return nc, in_maps


def kernel(x_batch, scores_x_batch, W0, b0, W1, b1, W2, b2, W3, b3, c):
    global LAST_RESULT
    nc, in_maps = build_for_inputs(x_batch, scores_x_batch, W0, b0, W1, b1,
                                   W2, b2, W3, b3, c)
    res = run_bass_kernel_spmd(nc, in_maps, core_ids=list(range(N_CORES)))
    LAST_RESULT = res
    return np.concatenate([r["out"] for r in res.results]).astype(np.float32)
